# revision 26
# baseline (speedup 1.0000x reference)
"""Trainium2 Bass kernel for BehlerG2-style symmetry functions.

Math (per (b,n,t) triple):
    s    = r_ij^2 + r_ik^2 + r_jk^2
    cut  = fc(r_ij)*fc(r_ik)*fc(r_jk),  fc(r) = 0.5*(cos(pi*r/6)+1)
    u    = 1 - s / (2*r_ij*r_ik)                  # = 1 - cos_theta
    W_e  = exp(-eta_e * s)                        # e in [0,16)
    v_p  = cut*mask * u^zeta_p                    # zeta = [1,2,4,8]
    f[b,n,e,z'] = sum_t W_e * v_p(z') * c_z'      # c = 2^(1-+zeta)

Mapping (per core, 128 atoms x 512 triples, SPMD over 8 cores):
  - inputs DMA'd in halves over sync+gpsimd queues
  - s in t-layout via accumulating PE transposes of the squared-r
    tensors; psS is read ONLY by the ACT exps (cross-engine PSUM reads
    serialize, so the u-chain uses an x-layout copy of s instead)
  - ACT: dummy Sin preloads the sin table at t~0; 3 Sin; exp-table
    load; 16 Exp (bf16 out, e-major contiguous) reading psS
  - u/w1 chain in x-layout on DVE (+ cut chain on Pool), then PE
    transposes u and w1; V ladder on DVE with z-MAJOR bf16 V so all
    writes are contiguous, 2^(1-zeta) folded into the z columns
  - PE: block-diag batched matmul, stationary V [t,(z,x)] bf16, moving
    W [t,(x,e)] bf16, PSUM accum over 4 chunks; e-split lo/hi halves so
    the lo matmuls overlap the exp phase; junk matmuls fill PE gaps to
    hold the clock up
  - diag extraction: PSUM->SBUF copies (DVE/ACT), CONTIGUOUS DMA to
    DRAM scratch, small strided diagonal gathers back (per e-half),
    permute+4^zeta scale on DVE, output DMA in two row-halves
"""

import math
import sys

import numpy as np

sys.path.insert(0, "/opt/trn_rl_repo")

_PROG_CACHE = {}

B, N, T = 4, 256, 512
E, Z = 16, 4
NCORES = 8
XA = (B * N) // NCORES  # atoms per core = 128
NG = 4  # atom groups per core (32 atoms each)
GSZ = XA // NG  # 32
NC_ = 4  # triple chunks (T/128)
EH = E // 2  # e-split half


def _np_reference(r_ij, r_ik, r_jk, mask_triples, etas, zetas):
    """Exact numpy fallback (matches reference.py) for unexpected params."""
    RC = 6.0

    def cut_fn(r):
        return np.where(r < RC, 0.5 * (np.cos(np.pi * r / RC) + 1.0), 0.0)

    r2 = r_ij**2 + r_ik**2 + r_jk**2
    cut = cut_fn(r_ij) * cut_fn(r_ik) * cut_fn(r_jk)
    radius = np.exp(-r2[..., None] * etas) * cut[..., None]
    cos_t = r2 / (2.0 * r_ij * r_ik)
    cos_t = np.where(mask_triples == 0, 0.0, cos_t)
    base = (1.0 - cos_t)[..., None] ** zetas
    ang = np.concatenate(
        [2.0 ** (1.0 - zetas) * base, 2.0 ** (1.0 + zetas) * base], axis=-1
    )
    f = np.einsum("bnt,bnte,bntz->bnez", mask_triples, radius, ang)
    return f.reshape(B, N, -1).astype(np.float32)


def build_core_kernel(tc, out_ap, in_aps, etas, zetas):
    """Emit one core's program into TileContext tc."""
    from contextlib import ExitStack

    import concourse.bass as bass
    import concourse.mybir as mybir
    from concourse import masks

    nc = tc.nc
    f32 = mybir.dt.float32
    bf16 = mybir.dt.bfloat16
    Alu = mybir.AluOpType
    Act = mybir.ActivationFunctionType

    H = T // 2  # input DMA half

    ctx = ExitStack()
    pool = ctx.enter_context(tc.tile_pool(name="main", bufs=1))
    psum = ctx.enter_context(tc.tile_pool(name="psum", bufs=1, space="PSUM"))
    dram = ctx.enter_context(tc.tile_pool(name="dram", bufs=1, space="DRAM"))
    scratch = dram.tile([NG, 2, 128, GSZ * EH], f32)

    # ---- x-layout tiles ----
    rstack = pool.tile([128, 3, T], f32)  # r_ij | r_ik | r_jk
    mask_n = pool.tile([128, T], f32)
    cstack = pool.tile([128, 3, T], f32)  # -cos(pi r / 6)
    sq1 = pool.tile([128, T], f32)
    sq2 = pool.tile([128, T], f32)
    sq3 = pool.tile([128, T], f32)
    s01 = pool.tile([128, T], f32)
    s_x = pool.tile([128, T], f32)
    prodm2 = pool.tile([128, T], f32)  # -2 r_ij r_ik
    rec = pool.tile([128, T], f32)
    m1 = pool.tile([128, T], f32)
    u_x = pool.tile([128, T], f32)
    cutst0 = pool.tile([128, T], f32)
    cutst1 = pool.tile([128, T], f32)
    cut12 = pool.tile([128, T], f32)
    c3t = pool.tile([128, T], f32)
    cut3 = pool.tile([128, T], f32)
    cutp = pool.tile([128, T], f32)  # cut * mask
    w1_x = pool.tile([128, T], f32)
    ident = pool.tile([128, 128], f32)
    dscr = pool.tile([128, 1], f32)  # dummy-activation scratch

    # t-layout tiles (shaped [c, g, x] so ladder slices match V's blocks)
    u_T = pool.tile([128, NC_, NG, GSZ], f32)
    u2_T = pool.tile([128, NC_, NG, GSZ], bf16)
    u4_T = pool.tile([128, NC_, NG, GSZ], bf16)
    # stationary V: per-(c,g) contiguous [Z, GSZ] block (z-major within the
    # block, so psG partition = 32z + x); ladder writes V[:, :, :, z, :]
    V = pool.tile([128, NC_, NG, Z, GSZ], bf16)
    # moving W: e-major so the 16 exp activations write contiguous slices
    W = pool.tile([128, E, NC_, 128], bf16)
    jl = pool.tile([128, 128], bf16)
    jr = pool.tile([128, 512], bf16)

    Gs = pool.tile([128, NG, 2, GSZ * EH], f32)  # PSUM drain staging
    out_tZ = pool.tile([128, Z, E], f32)  # gathered diagonal, z-major
    out_t = pool.tile([128, E, 2 * Z], f32)  # final output tile
    sc4 = pool.tile([128, E, Z], f32)  # 4**zeta pattern

    psS = psum.tile([128, NC_, 128], f32)
    psU = psum.tile([128, NC_, NG, GSZ], f32)
    psW1 = psum.tile([128, NC_, NG, GSZ], f32)
    psJ = psum.tile([128, 512], f32)
    psG = [psum.tile([128, 2, GSZ * EH], f32, name=f"psG{g}") for g in range(NG)]

    # ---- input DMAs: halves over sync + gpsimd queues ----
    nc.sync.dma_start(rstack[:, 0, 0:H], in_aps["r_ij"][:, 0:H])
    nc.sync.dma_start(rstack[:, 0, H:T], in_aps["r_ij"][:, H:T])
    nc.scalar.dma_start(rstack[:, 1, 0:H], in_aps["r_ik"][:, 0:H])
    nc.scalar.dma_start(rstack[:, 1, H:T], in_aps["r_ik"][:, H:T])
    nc.gpsimd.dma_start(rstack[:, 2, 0:H], in_aps["r_jk"][:, 0:H])
    nc.gpsimd.dma_start(rstack[:, 2, H:T], in_aps["r_jk"][:, H:T])

    # ---- ACT: preload sin table via dummy (scratch tile, read by nothing) --
    nc.scalar.activation(dscr[:], dscr[:], Act.Sin)

    # ---- constants (gpsimd queue, after its DMA issues) ----
    neg_half_pi = pool.tile([128, 1], f32)
    nc.gpsimd.memset(jl[:], 0.5)
    nc.gpsimd.memset(jr[:], 0.5)
    nc.gpsimd.memset(neg_half_pi[:], -math.pi / 2.0)
    masks.make_identity(nc, ident[:])
    for zi in range(Z):
        nc.gpsimd.memset(sc4[:, :, zi], float(4.0 ** float(zetas[zi])))
    nc.gpsimd.dma_start(mask_n[:, 0:H], in_aps["mask_triples"][:, 0:H])
    nc.gpsimd.dma_start(mask_n[:, H:T], in_aps["mask_triples"][:, H:T])

    # ---- PE: warm-up junk (batch 1) ----
    def junk(n, start=True):
        for k in range(n):
            nc.tensor.matmul(
                psJ[:], jl[:], jr[:], start=(k == 0), stop=(k == n - 1)
            )

    junk(4)

    # ---- squares (DVE, halves) then full-width adds ----
    for h in range(2):
        sl = slice(h * H, (h + 1) * H)
        nc.vector.tensor_mul(sq1[:, sl], rstack[:, 0, sl], rstack[:, 0, sl])
        nc.vector.tensor_mul(sq2[:, sl], rstack[:, 1, sl], rstack[:, 1, sl])
        nc.vector.tensor_mul(sq3[:, sl], rstack[:, 2, sl], rstack[:, 2, sl])
    nc.vector.tensor_add(s01[:], sq1[:], sq2[:])

    # ---- PE: s in t-layout via accumulating transposes (exps' sole input) --
    for c in range(NC_):
        cs = slice(c * 128, (c + 1) * 128)
        for j, sq in enumerate((sq1, sq2, sq3)):
            nc.tensor.matmul(
                psS[:, c, :], sq[:, cs], ident[:],
                is_transpose=True, start=(j == 0), stop=(j == 2),
            )

    # ---- ACT: 3 cutoff cosines;  -cos(pi r/6) = sin(pi/6 * r - pi/2) ----
    for i in range(3):
        nc.scalar.activation(
            cstack[:, i, :], rstack[:, i, :], Act.Sin,
            bias=neg_half_pi[:], scale=math.pi / 6.0,
        )

    # ---- ACT: 16 exps -> bf16 W, e-major contiguous writes ----
    for e in range(E):
        nc.scalar.activation(W[:, e], psS[:], Act.Exp, scale=-float(etas[e]))

    # ---- DVE: u chain (x-layout; psS stays ACT-only) ----
    nc.vector.tensor_mul(prodm2[:], rstack[:, 0, :], rstack[:, 1, :])
    nc.vector.tensor_add(s_x[:], s01[:], sq3[:])
    nc.vector.reciprocal_approx_fast(rec[:], prodm2[:])
    nc.vector.tensor_mul(m1[:], s_x[:], rec[:])  # s / (r_ij r_ik)
    nc.vector.tensor_scalar(u_x[:], m1[:], -0.5, 1.0, Alu.mult, Alu.add)

    # ---- cut chain: TS pieces on Pool, products on DVE ----
    # cstack holds -cos; fc = 0.5 - 0.5*cstack
    nc.gpsimd.tensor_scalar(cutst0[:], cstack[:, 0], -0.5, 0.5, Alu.mult, Alu.add)
    nc.gpsimd.tensor_scalar(cutst1[:], cstack[:, 1], -0.5, 0.5, Alu.mult, Alu.add)
    nc.gpsimd.tensor_scalar(c3t[:], cstack[:, 2], -0.5, 0.5, Alu.mult, Alu.add)
    nc.gpsimd.tensor_mul(cut12[:], cutst0[:], cutst1[:])
    nc.vector.tensor_mul(cut3[:], c3t[:], mask_n[:])

    # ---- PE: junk staggered through the exp phase (junk e's moving operand
    # is W[:, e], runnable only after exp e; placed by expected readiness
    # since the PE queue is in-order) ----
    def wjunk(es):
        for e in es:
            nc.tensor.matmul(psJ[:], jl[:], W[:, e].opt(), start=True, stop=True)

    wjunk(range(0, 6))

    # ---- PE: transpose u ----
    for c in range(NC_):
        nc.tensor.transpose(psU[:, c].opt(), u_x[:, c * 128 : (c + 1) * 128], ident[:])
    wjunk(range(6, 8))

    # ---- DVE: u powers (from psU) + w1 chain ----
    nc.vector.tensor_copy(u_T[:], psU[:])
    nc.vector.tensor_mul(cutp[:], cut12[:], cut3[:])
    nc.vector.tensor_mul(w1_x[:], cutp[:], u_x[:])
    nc.vector.tensor_mul(u2_T[:], u_T[:], u_T[:])
    nc.vector.tensor_mul(u4_T[:], u2_T[:], u2_T[:])

    # ---- PE: transpose w1 ----
    for c in range(NC_):
        nc.tensor.transpose(psW1[:, c].opt(), w1_x[:, c * 128 : (c + 1) * 128], ident[:])
    wjunk(range(8, 15))

    # ---- DVE: V ladder ----
    nc.vector.tensor_copy(V[:, :, :, 0, :], psW1[:])
    nc.vector.scalar_tensor_tensor(
        V[:, :, :, 1, :], psW1[:], 0.5, u_T[:], Alu.mult, Alu.mult
    )
    nc.vector.scalar_tensor_tensor(
        V[:, :, :, 2, :], V[:, :, :, 1, :], 0.25, u2_T[:], Alu.mult, Alu.mult
    )
    nc.vector.scalar_tensor_tensor(
        V[:, :, :, 3, :], V[:, :, :, 2, :], 0.0625, u4_T[:], Alu.mult, Alu.mult
    )

    # ---- PE: lo-half matmuls (e 0:8), overlapping the exp phase ----
    lo_dma = [nc.sync, nc.gpsimd, nc.sync, nc.gpsimd]
    for g in range(NG):
        gs = slice(g * GSZ, (g + 1) * GSZ)
        for c in range(NC_):
            nc.tensor.matmul(
                psG[g][:, 0], V[:, c, g].opt(),
                W[:, 0:EH, c, gs].transpose([0, 2, 1]),
                start=(c == 0), stop=(c == NC_ - 1),
            )
        nc.vector.tensor_copy(Gs[:, g, 0], psG[g][:, 0])
        lo_dma[g].dma_start(scratch[g, 0], Gs[:, g, 0])

    # ---- PE: hi-half matmuls (e 8:16), g-outer so drains start early ----
    for g in range(NG):
        gs = slice(g * GSZ, (g + 1) * GSZ)
        for c in range(NC_):
            nc.tensor.matmul(
                psG[g][:, 1], V[:, c, g].opt(),
                W[:, EH:E, c, gs].transpose([0, 2, 1]),
                start=(c == 0), stop=(c == NC_ - 1),
            )

    # ---- lo gathers (hidden under the hi-MM phase; gpsimd is fine) ----
    def diag_src(g, half):
        sc = scratch[g, half]
        return bass.AP(
            sc.tensor, sc.offset,
            [[GSZ * EH + EH, GSZ], [GSZ * GSZ * EH, Z], [1, EH]],
        )

    for g in range(NG):
        gs = slice(g * GSZ, (g + 1) * GSZ)
        nc.gpsimd.dma_start(out_tZ[gs, :, 0:EH], diag_src(g, 0))

    # ---- per-group tail chains on hw queues: g0/g2 sync, g1/g3 scalar ----
    hi_copy = [nc.scalar.copy, nc.vector.tensor_copy, nc.vector.tensor_copy,
               nc.scalar.copy]
    for g in range(NG):
        hi_copy[g](Gs[:, g, 1], psG[g][:, 1])
    for g in (0, 2):
        nc.sync.dma_start(scratch[g, 1], Gs[:, g, 1])
    for g in (1, 3):
        nc.scalar.dma_start(scratch[g, 1], Gs[:, g, 1])
    for g in (0, 2):
        gs = slice(g * GSZ, (g + 1) * GSZ)
        nc.sync.dma_start(out_tZ[gs, :, EH:E], diag_src(g, 1))
    for g in (1, 3):
        gs = slice(g * GSZ, (g + 1) * GSZ)
        nc.scalar.dma_start(out_tZ[gs, :, EH:E], diag_src(g, 1))
    # permute (z,e)->(e,z) + 4^zeta scale, per group
    for g in range(NG):
        gs = slice(g * GSZ, (g + 1) * GSZ)
        nc.vector.tensor_copy(out_t[gs, :, 0:Z], out_tZ[gs].transpose([0, 2, 1]))
        nc.vector.tensor_mul(out_t[gs, :, Z : 2 * Z], out_t[gs, :, 0:Z], sc4[gs])
    for g in (0, 2):
        gs = slice(g * GSZ, (g + 1) * GSZ)
        nc.sync.dma_start(out_ap[gs, :], out_t[gs].opt())
    for g in (1, 3):
        gs = slice(g * GSZ, (g + 1) * GSZ)
        nc.scalar.dma_start(out_ap[gs, :], out_t[gs].opt())
    ctx.close()


def _build_program(etas, zetas):
    import concourse.bacc as bacc
    import concourse.mybir as mybir
    import concourse.tile as tile

    f32 = mybir.dt.float32
    nc = bacc.Bacc("TRN2", target_bir_lowering=False, debug=False, num_devices=NCORES)

    in_aps = {}
    for name in ("r_ij", "r_ik", "r_jk", "mask_triples"):
        in_aps[name] = nc.declare_dram_parameter(name, [XA, T], f32, isOutput=False).ap()
    out_ap = nc.declare_dram_parameter("out", [XA, E * 2 * Z], f32, isOutput=True).ap()

    with tile.TileContext(nc) as tc:
        build_core_kernel(tc, out_ap, in_aps, etas, zetas)
    nc.compile()
    return nc


def _get_program(etas, zetas):
    key = (tuple(float(x) for x in etas), tuple(float(x) for x in zetas))
    if key not in _PROG_CACHE:
        _PROG_CACHE[key] = _build_program(etas, zetas)
    return _PROG_CACHE[key]


def kernel(r_ij, r_ik, r_jk, mask_triples, etas, zetas):
    etas = np.asarray(etas, np.float32)
    zetas = np.asarray(zetas, np.float32)
    args = dict(r_ij=r_ij, r_ik=r_ik, r_jk=r_jk, mask_triples=mask_triples)

    # fast path requires zeta = [1, 2, 4, 8] (powers computed by squaring)
    if (
        tuple(zetas.tolist()) != (1.0, 2.0, 4.0, 8.0)
        or r_ij.shape != (B, N, T)
        or float(np.max(np.abs([r_ij.max(), r_ik.max(), r_jk.max()]))) >= 6.0
    ):
        return _np_reference(
            np.asarray(r_ij), np.asarray(r_ik), np.asarray(r_jk),
            np.asarray(mask_triples), etas, zetas,
        )

    from concourse.bass_utils import run_bass_kernel_spmd

    nc = _get_program(etas, zetas)
    flat = {k: np.ascontiguousarray(np.asarray(v, np.float32).reshape(B * N, T))
            for k, v in args.items()}
    in_maps = [
        {k: v[c * XA : (c + 1) * XA] for k, v in flat.items()} for c in range(NCORES)
    ]
    res = run_bass_kernel_spmd(nc, in_maps, list(range(NCORES)))
    out = np.concatenate([res.results[c]["out"] for c in range(NCORES)], axis=0)
    return out.reshape(B, N, E * 2 * Z).astype(np.float32)


# revision 27
# speedup vs baseline: 1.0380x; 1.0380x over previous
"""Trainium2 Bass kernel for BehlerG2-style symmetry functions.

Math (per (b,n,t) triple):
    s    = r_ij^2 + r_ik^2 + r_jk^2
    cut  = fc(r_ij)*fc(r_ik)*fc(r_jk),  fc(r) = 0.5*(cos(pi*r/6)+1)
    u    = 1 - s / (2*r_ij*r_ik)                  # = 1 - cos_theta
    W_e  = exp(-eta_e * s)                        # e in [0,16)
    v_p  = cut*mask * u^zeta_p                    # zeta = [1,2,4,8]
    f[b,n,e,z'] = sum_t W_e * v_p(z') * c_z'      # c = 2^(1-+zeta)

Mapping (per core, 128 atoms x 512 triples, SPMD over 8 cores):
  - inputs DMA'd in halves over sync+gpsimd queues
  - s in t-layout via accumulating PE transposes of the squared-r
    tensors; psS is read ONLY by the ACT exps (cross-engine PSUM reads
    serialize, so the u-chain uses an x-layout copy of s instead)
  - ACT: dummy Sin preloads the sin table at t~0; 3 Sin; exp-table
    load; 16 Exp (bf16 out, e-major contiguous) reading psS
  - u/w1 chain in x-layout on DVE (+ cut chain on Pool), then PE
    transposes u and w1; V ladder on DVE with z-MAJOR bf16 V so all
    writes are contiguous, 2^(1-zeta) folded into the z columns
  - PE: block-diag batched matmul, stationary V [t,(z,x)] bf16, moving
    W [t,(x,e)] bf16, PSUM accum over 4 chunks; e-split lo/hi halves so
    the lo matmuls overlap the exp phase; junk matmuls fill PE gaps to
    hold the clock up
  - diag extraction: PSUM->SBUF copies (DVE/ACT), CONTIGUOUS DMA to
    DRAM scratch, small strided diagonal gathers back (per e-half),
    permute+4^zeta scale on DVE, output DMA in two row-halves
"""

import math
import sys

import numpy as np

sys.path.insert(0, "/opt/trn_rl_repo")

_PROG_CACHE = {}

B, N, T = 4, 256, 512
E, Z = 16, 4
NCORES = 8
XA = (B * N) // NCORES  # atoms per core = 128
NG = 4  # atom groups per core (32 atoms each)
GSZ = XA // NG  # 32
NC_ = 4  # triple chunks (T/128)
EH = E // 2  # e-split half


def _np_reference(r_ij, r_ik, r_jk, mask_triples, etas, zetas):
    """Exact numpy fallback (matches reference.py) for unexpected params."""
    RC = 6.0

    def cut_fn(r):
        return np.where(r < RC, 0.5 * (np.cos(np.pi * r / RC) + 1.0), 0.0)

    r2 = r_ij**2 + r_ik**2 + r_jk**2
    cut = cut_fn(r_ij) * cut_fn(r_ik) * cut_fn(r_jk)
    radius = np.exp(-r2[..., None] * etas) * cut[..., None]
    cos_t = r2 / (2.0 * r_ij * r_ik)
    cos_t = np.where(mask_triples == 0, 0.0, cos_t)
    base = (1.0 - cos_t)[..., None] ** zetas
    ang = np.concatenate(
        [2.0 ** (1.0 - zetas) * base, 2.0 ** (1.0 + zetas) * base], axis=-1
    )
    f = np.einsum("bnt,bnte,bntz->bnez", mask_triples, radius, ang)
    return f.reshape(B, N, -1).astype(np.float32)


def build_core_kernel(tc, out_ap, in_aps, etas, zetas):
    """Emit one core's program into TileContext tc."""
    from contextlib import ExitStack

    import concourse.bass as bass
    import concourse.mybir as mybir
    from concourse import masks

    nc = tc.nc
    f32 = mybir.dt.float32
    bf16 = mybir.dt.bfloat16
    Alu = mybir.AluOpType
    Act = mybir.ActivationFunctionType

    H = T // 2  # input DMA half

    ctx = ExitStack()
    pool = ctx.enter_context(tc.tile_pool(name="main", bufs=1))
    psum = ctx.enter_context(tc.tile_pool(name="psum", bufs=1, space="PSUM"))
    dram = ctx.enter_context(tc.tile_pool(name="dram", bufs=1, space="DRAM"))
    scratch = dram.tile([NG, 2, 128, GSZ * EH], f32)

    # ---- x-layout tiles ----
    rstack = pool.tile([128, 3, T], f32)  # r_ij | r_ik | r_jk
    mask_n = pool.tile([128, T], f32)
    cstack = pool.tile([128, 3, T], f32)  # -cos(pi r / 6)
    sq1 = pool.tile([128, T], f32)
    sq2 = pool.tile([128, T], f32)
    sq3 = pool.tile([128, T], f32)
    s01 = pool.tile([128, T], f32)
    s_x = pool.tile([128, T], f32)
    prodm2 = pool.tile([128, T], f32)  # -2 r_ij r_ik
    rec = pool.tile([128, T], f32)
    m1 = pool.tile([128, T], f32)
    u_x = pool.tile([128, T], f32)
    cutst0 = pool.tile([128, T], f32)
    cutst1 = pool.tile([128, T], f32)
    cut12 = pool.tile([128, T], f32)
    c3t = pool.tile([128, T], f32)
    cut3 = pool.tile([128, T], f32)
    cutp = pool.tile([128, T], f32)  # cut * mask
    w1_x = pool.tile([128, T], f32)
    ident = pool.tile([128, 128], f32)
    dscr = pool.tile([128, 1], f32)  # dummy-activation scratch

    # t-layout tiles (shaped [c, g, x] so ladder slices match V's blocks)
    u_T = pool.tile([128, NC_, NG, GSZ], f32)
    u2_T = pool.tile([128, NC_, NG, GSZ], bf16)
    u4_T = pool.tile([128, NC_, NG, GSZ], bf16)
    # stationary V: per-(c,g) contiguous [Z, GSZ] block (z-major within the
    # block, so psG partition = 32z + x); ladder writes V[:, :, :, z, :]
    V = pool.tile([128, NC_, NG, Z, GSZ], bf16)
    # moving W: e-major so the 16 exp activations write contiguous slices
    W = pool.tile([128, E, NC_, 128], bf16)
    jl = pool.tile([128, 128], bf16)
    jr = pool.tile([128, 512], bf16)

    Gs = pool.tile([128, NG, 2, GSZ * EH], f32)  # PSUM drain staging
    out_tZ = pool.tile([128, Z, E], f32)  # gathered diagonal, z-major
    out_t = pool.tile([128, E, 2 * Z], f32)  # final output tile
    sc4 = pool.tile([128, E, Z], f32)  # 4**zeta pattern

    psS = psum.tile([128, NC_, 128], f32)
    psU = psum.tile([128, NC_, NG, GSZ], f32)
    psW1 = psum.tile([128, NC_, NG, GSZ], f32)
    psJ = psum.tile([128, 512], f32)
    psG = [psum.tile([128, 2, GSZ * EH], f32, name=f"psG{g}") for g in range(NG)]

    # ---- input DMAs: halves over sync + gpsimd queues ----
    nc.sync.dma_start(rstack[:, 0, 0:H], in_aps["r_ij"][:, 0:H])
    nc.sync.dma_start(rstack[:, 0, H:T], in_aps["r_ij"][:, H:T])
    nc.scalar.dma_start(rstack[:, 1, 0:H], in_aps["r_ik"][:, 0:H])
    nc.scalar.dma_start(rstack[:, 1, H:T], in_aps["r_ik"][:, H:T])
    nc.gpsimd.dma_start(rstack[:, 2, 0:H], in_aps["r_jk"][:, 0:H])
    nc.gpsimd.dma_start(rstack[:, 2, H:T], in_aps["r_jk"][:, H:T])

    # ---- ACT: preload sin table via dummy (scratch tile, read by nothing) --
    nc.scalar.activation(dscr[:], dscr[:], Act.Sin)

    # ---- constants (gpsimd queue, after its DMA issues) ----
    neg_half_pi = pool.tile([128, 1], f32)
    nc.gpsimd.memset(jl[:], 0.5)
    nc.gpsimd.memset(jr[:], 0.5)
    nc.gpsimd.memset(neg_half_pi[:], -math.pi / 2.0)
    masks.make_identity(nc, ident[:])
    for zi in range(Z):
        nc.gpsimd.memset(sc4[:, :, zi], float(4.0 ** float(zetas[zi])))
    nc.gpsimd.dma_start(mask_n[:, 0:H], in_aps["mask_triples"][:, 0:H])
    nc.gpsimd.dma_start(mask_n[:, H:T], in_aps["mask_triples"][:, H:T])

    # ---- PE: warm-up junk (batch 1) ----
    def junk(n, start=True):
        for k in range(n):
            nc.tensor.matmul(
                psJ[:], jl[:], jr[:], start=(k == 0), stop=(k == n - 1)
            )

    junk(4)

    # ---- squares (DVE, halves) then full-width adds ----
    for h in range(2):
        sl = slice(h * H, (h + 1) * H)
        nc.vector.tensor_mul(sq1[:, sl], rstack[:, 0, sl], rstack[:, 0, sl])
        nc.vector.tensor_mul(sq2[:, sl], rstack[:, 1, sl], rstack[:, 1, sl])
        nc.vector.tensor_mul(sq3[:, sl], rstack[:, 2, sl], rstack[:, 2, sl])
    nc.vector.tensor_add(s01[:], sq1[:], sq2[:])

    # ---- PE: s in t-layout via accumulating transposes (exps' sole input) --
    for c in range(NC_):
        cs = slice(c * 128, (c + 1) * 128)
        for j, sq in enumerate((sq1, sq2, sq3)):
            nc.tensor.matmul(
                psS[:, c, :], sq[:, cs], ident[:],
                is_transpose=True, start=(j == 0), stop=(j == 2),
            )

    # ---- ACT: 3 cutoff cosines;  -cos(pi r/6) = sin(pi/6 * r - pi/2) ----
    for i in range(3):
        nc.scalar.activation(
            cstack[:, i, :], rstack[:, i, :], Act.Sin,
            bias=neg_half_pi[:], scale=math.pi / 6.0,
        )

    # ---- ACT: 16 exps -> bf16 W, e-major contiguous writes ----
    for e in range(E):
        nc.scalar.activation(W[:, e], psS[:], Act.Exp, scale=-float(etas[e]))

    # ---- DVE: u chain (x-layout; psS stays ACT-only) ----
    nc.vector.tensor_mul(prodm2[:], rstack[:, 0, :], rstack[:, 1, :])
    nc.vector.tensor_add(s_x[:], s01[:], sq3[:])
    nc.vector.reciprocal_approx_fast(rec[:], prodm2[:])
    nc.vector.tensor_mul(m1[:], s_x[:], rec[:])  # s / (r_ij r_ik)
    nc.vector.tensor_scalar(u_x[:], m1[:], -0.5, 1.0, Alu.mult, Alu.add)

    # ---- cut chain: TS pieces on Pool, products on DVE ----
    # cstack holds -cos; fc = 0.5 - 0.5*cstack
    nc.gpsimd.tensor_scalar(cutst0[:], cstack[:, 0], -0.5, 0.5, Alu.mult, Alu.add)
    nc.gpsimd.tensor_scalar(cutst1[:], cstack[:, 1], -0.5, 0.5, Alu.mult, Alu.add)
    nc.gpsimd.tensor_scalar(c3t[:], cstack[:, 2], -0.5, 0.5, Alu.mult, Alu.add)
    nc.gpsimd.tensor_mul(cut12[:], cutst0[:], cutst1[:])
    nc.vector.tensor_mul(cut3[:], c3t[:], mask_n[:])

    # ---- PE: junk staggered through the exp phase (junk e's moving operand
    # is W[:, e], runnable only after exp e; placed by expected readiness
    # since the PE queue is in-order) ----
    def wjunk(es, double=()):
        for e in es:
            nc.tensor.matmul(psJ[:], jl[:], W[:, e].opt(), start=True, stop=True)
            if e in double:
                nc.tensor.matmul(psJ[:], jl[:], W[:, e].opt(), start=True, stop=True)

    wjunk(range(0, 6), double=(2, 3, 4, 5))

    # ---- PE: transpose u ----
    for c in range(NC_):
        nc.tensor.transpose(psU[:, c].opt(), u_x[:, c * 128 : (c + 1) * 128], ident[:])
    wjunk(range(6, 8))

    # ---- DVE: u powers (from psU) + w1 chain ----
    nc.vector.tensor_copy(u_T[:], psU[:])
    nc.vector.tensor_mul(cutp[:], cut12[:], cut3[:])
    nc.vector.tensor_mul(w1_x[:], cutp[:], u_x[:])
    nc.vector.tensor_mul(u2_T[:], u_T[:], u_T[:])
    nc.vector.tensor_mul(u4_T[:], u2_T[:], u2_T[:])

    # ---- PE: transpose w1 ----
    for c in range(NC_):
        nc.tensor.transpose(psW1[:, c].opt(), w1_x[:, c * 128 : (c + 1) * 128], ident[:])
    wjunk(range(8, 15), double=(9, 10, 11, 12))

    # ---- DVE: V ladder ----
    nc.vector.tensor_copy(V[:, :, :, 0, :], psW1[:])
    nc.vector.scalar_tensor_tensor(
        V[:, :, :, 1, :], psW1[:], 0.5, u_T[:], Alu.mult, Alu.mult
    )
    nc.vector.scalar_tensor_tensor(
        V[:, :, :, 2, :], V[:, :, :, 1, :], 0.25, u2_T[:], Alu.mult, Alu.mult
    )
    nc.vector.scalar_tensor_tensor(
        V[:, :, :, 3, :], V[:, :, :, 2, :], 0.0625, u4_T[:], Alu.mult, Alu.mult
    )

    # ---- PE: lo-half matmuls (e 0:8), overlapping the exp phase ----
    lo_dma = [nc.sync, nc.gpsimd, nc.sync, nc.gpsimd]
    for g in range(NG):
        gs = slice(g * GSZ, (g + 1) * GSZ)
        for c in range(NC_):
            nc.tensor.matmul(
                psG[g][:, 0], V[:, c, g].opt(),
                W[:, 0:EH, c, gs].transpose([0, 2, 1]),
                start=(c == 0), stop=(c == NC_ - 1),
            )
        nc.vector.tensor_copy(Gs[:, g, 0], psG[g][:, 0])
        lo_dma[g].dma_start(scratch[g, 0], Gs[:, g, 0])

    # ---- PE: hi-half matmuls (e 8:16), g-outer so drains start early ----
    for g in range(NG):
        gs = slice(g * GSZ, (g + 1) * GSZ)
        for c in range(NC_):
            nc.tensor.matmul(
                psG[g][:, 1], V[:, c, g].opt(),
                W[:, EH:E, c, gs].transpose([0, 2, 1]),
                start=(c == 0), stop=(c == NC_ - 1),
            )

    # ---- lo gathers (hidden under the hi-MM phase; gpsimd is fine) ----
    def diag_src(g, half):
        sc = scratch[g, half]
        return bass.AP(
            sc.tensor, sc.offset,
            [[GSZ * EH + EH, GSZ], [GSZ * GSZ * EH, Z], [1, EH]],
        )

    for g in range(NG):
        gs = slice(g * GSZ, (g + 1) * GSZ)
        nc.gpsimd.dma_start(out_tZ[gs, :, 0:EH], diag_src(g, 0))

    # ---- per-group tail chains on hw queues: g0/g2 sync, g1/g3 scalar ----
    hi_copy = [nc.scalar.copy, nc.vector.tensor_copy, nc.vector.tensor_copy,
               nc.scalar.copy]
    for g in range(NG):
        hi_copy[g](Gs[:, g, 1], psG[g][:, 1])
    for g in (0, 2):
        nc.sync.dma_start(scratch[g, 1], Gs[:, g, 1])
    for g in (1, 3):
        nc.scalar.dma_start(scratch[g, 1], Gs[:, g, 1])
    for g in (0, 2):
        gs = slice(g * GSZ, (g + 1) * GSZ)
        nc.sync.dma_start(out_tZ[gs, :, EH:E], diag_src(g, 1))
    for g in (1, 3):
        gs = slice(g * GSZ, (g + 1) * GSZ)
        nc.scalar.dma_start(out_tZ[gs, :, EH:E], diag_src(g, 1))
    # permute (z,e)->(e,z) + 4^zeta scale, per group
    for g in range(NG):
        gs = slice(g * GSZ, (g + 1) * GSZ)
        nc.vector.tensor_copy(out_t[gs, :, 0:Z], out_tZ[gs].transpose([0, 2, 1]))
        nc.vector.tensor_mul(out_t[gs, :, Z : 2 * Z], out_t[gs, :, 0:Z], sc4[gs])
    nc.sync.dma_start(out_ap[0:64, :], out_t[0:64].opt())
    nc.scalar.dma_start(out_ap[64:128, :], out_t[64:128].opt())
    ctx.close()


def _build_program(etas, zetas):
    import concourse.bacc as bacc
    import concourse.mybir as mybir
    import concourse.tile as tile

    f32 = mybir.dt.float32
    nc = bacc.Bacc("TRN2", target_bir_lowering=False, debug=False, num_devices=NCORES)

    in_aps = {}
    for name in ("r_ij", "r_ik", "r_jk", "mask_triples"):
        in_aps[name] = nc.declare_dram_parameter(name, [XA, T], f32, isOutput=False).ap()
    out_ap = nc.declare_dram_parameter("out", [XA, E * 2 * Z], f32, isOutput=True).ap()

    with tile.TileContext(nc) as tc:
        build_core_kernel(tc, out_ap, in_aps, etas, zetas)
    nc.compile()
    return nc


def _get_program(etas, zetas):
    key = (tuple(float(x) for x in etas), tuple(float(x) for x in zetas))
    if key not in _PROG_CACHE:
        _PROG_CACHE[key] = _build_program(etas, zetas)
    return _PROG_CACHE[key]


def kernel(r_ij, r_ik, r_jk, mask_triples, etas, zetas):
    etas = np.asarray(etas, np.float32)
    zetas = np.asarray(zetas, np.float32)
    args = dict(r_ij=r_ij, r_ik=r_ik, r_jk=r_jk, mask_triples=mask_triples)

    # fast path requires zeta = [1, 2, 4, 8] (powers computed by squaring)
    if (
        tuple(zetas.tolist()) != (1.0, 2.0, 4.0, 8.0)
        or r_ij.shape != (B, N, T)
        or float(np.max(np.abs([r_ij.max(), r_ik.max(), r_jk.max()]))) >= 6.0
    ):
        return _np_reference(
            np.asarray(r_ij), np.asarray(r_ik), np.asarray(r_jk),
            np.asarray(mask_triples), etas, zetas,
        )

    from concourse.bass_utils import run_bass_kernel_spmd

    nc = _get_program(etas, zetas)
    flat = {k: np.ascontiguousarray(np.asarray(v, np.float32).reshape(B * N, T))
            for k, v in args.items()}
    in_maps = [
        {k: v[c * XA : (c + 1) * XA] for k, v in flat.items()} for c in range(NCORES)
    ]
    res = run_bass_kernel_spmd(nc, in_maps, list(range(NCORES)))
    out = np.concatenate([res.results[c]["out"] for c in range(NCORES)], axis=0)
    return out.reshape(B, N, E * 2 * Z).astype(np.float32)
